# revision 27
# baseline (speedup 1.0000x reference)
"""Trainium2 Bass kernel for CategoricalEntropyRegLoss.

Math: both loss terms factor so the [B,B] pairwise matrices are never built.
After L2-normalization sq_j = 1 exactly, so the m*sq column collapses into
the mask column (a = M, se = e, Psq = Pbar, Lsq = L):

  S = sum_{jk} m_j m_k feat_dists * target_dists
    = 2*[ M*e - Pbar.Lbar - Fe.F~ + <U,V> ]
  div = -S / (D * M * (M-1))          (D = 2)
  tight*M = M - sum_s ||seg_sum_s||^2 / max(cnt_s,1)

One matmul per core:
  out[1154, 257] = ext_seg^T @ ext_feat
  ext_seg  = [ onehot(code) | LQ | P | 1 | E ]      (B x 1154)
  ext_feat = [ m*fn | m ]                           (B x 257)
Matmul operands are bf16, the AllReduce payload fp16 (counts <= 2048
exact; measured end-to-end rel err ~8.7e-3 against a 2e-2 gate).
argmax/code stay fp32 so segment assignment matches the reference.

Schedule: row norms on DVE (no front Square table), ACT chain is
Sqrt-prefetch / sqrt / feature copies / Ln / Square-prefetch — tables
load off the critical path. The matmul runs k-chunk-outer with m-tiles
0-7 accumulating in all 8 PSUM banks; the two stats tiles run as a
trailing wave in recycled banks. The AllReduce buffer is fully packed:
m-tiles 0-8 contiguous + the two [1,257] tail rows flattened into a
[128,5] block (no 126-row zero pad). Every core runs the epilogue
redundantly, segment squares split DVE/ACT.
"""

import numpy as np

B = 4096
FD = 256
C = 32
D = 2
NSEG = C ** D          # 1024
NCORES = 8
RB = B // NCORES       # 512 rows per core
KT = RB // 128         # 4 k-chunks of 128 rows
EF = FD + 1            # 257: [mfn | m]
ES = NSEG + 2 * D * C + 2   # 1154
NST = 2 * D * C + 2         # 130 stats columns
TW = 9 * EF                 # 2313: flat width of m-tiles 0-8
TAILW = 5                   # tail block: [128, 5] holds 2x320 flattened
BW = TW + TAILW             # 2318 total AR columns per partition

_compiled = {}


def _build_bass():
    from contextlib import ExitStack
    import concourse.bass as bass
    import concourse.bacc as bacc
    import concourse.tile as tile
    from concourse import mybir

    from concourse.tile import add_dep_helper

    f32 = mybir.dt.float32
    bf16 = mybir.dt.bfloat16
    fp16 = mybir.dt.float16
    Alu = mybir.AluOpType
    Act = mybir.ActivationFunctionType
    Ax = mybir.AxisListType

    nc = bacc.Bacc(num_devices=NCORES)

    feat = nc.dram_tensor("features", [RB, FD], f32, kind="ExternalInput")
    targ = nc.dram_tensor("targets", [RB, D * C], f32, kind="ExternalInput")
    maskf = nc.dram_tensor("maskf", [RB, 1], f32, kind="ExternalInput")
    outd = nc.dram_tensor("out", [8], f32, kind="ExternalOutput")

    with ExitStack() as ctx:
        tc = ctx.enter_context(tile.TileContext(nc))
        consts = ctx.enter_context(tc.tile_pool(name="consts", bufs=1))
        keep = ctx.enter_context(tc.tile_pool(name="keep", bufs=1))
        psum = ctx.enter_context(tc.tile_pool(name="psum", bufs=1, space="PSUM"))
        dram = ctx.enter_context(tc.tile_pool(name="dram", bufs=1, space="DRAM"))

        # ---------------- constants ----------------
        ones128 = consts.tile([128, 1], f32)
        nc.vector.memset(ones128[:], 1.0)

        # ACT Sqrt-table prefetch while the input DMAs are in flight
        pre = consts.tile([1, 1], f32)
        pre_act = nc.scalar.sqrt(pre[:], ones128[0:1, 0:1])

        # flat-packed AllReduce buffer (fp16, no pad rows)
        inbounce = dram.tile([128, BW], fp16, name="inbounce")
        outbounce = dram.tile([128, BW], fp16, name="outbounce",
                              addr_space="Shared")

        # ---- batched input loads spread over queues ----
        tbig = keep.tile([128, KT, D * C], f32, name="tbig")
        nc.scalar.dma_start(
            out=tbig[:], in_=targ[:, :].rearrange("(a p) f -> p a f", p=128))
        mkbig = keep.tile([128, KT, 1], f32, name="mkbig")
        nc.scalar.dma_start(
            out=mkbig[:], in_=maskf[:, :].rearrange("(a p) f -> p a f", p=128))
        xbig0 = keep.tile([128, 2, FD], f32, name="xbig0")
        nc.sync.dma_start(
            out=xbig0[:],
            in_=feat[0:256, :].rearrange("(a p) f -> p a f", p=128))
        xbig1 = keep.tile([128, 2, FD], f32, name="xbig1")
        nc.gpsimd.dma_start(
            out=xbig1[:],
            in_=feat[256:512, :].rearrange("(a p) f -> p a f", p=128))

        def xchunk(kc):
            return xbig0[:, kc, :] if kc < 2 else xbig1[:, kc - 2, :]

        # iotas after the gpsimd input DMA trigger
        iota1024 = consts.tile([128, NSEG], f32)
        nc.gpsimd.iota(iota1024[:], [[1, NSEG]], channel_multiplier=0,
                       allow_small_or_imprecise_dtypes=True)
        # biota[j] = 32 - j  (for first-argmax via reduce_max)
        biota = consts.tile([128, C], f32)
        nc.gpsimd.iota(biota[:], [[-1, C]], base=C, channel_multiplier=0,
                       allow_small_or_imprecise_dtypes=True)

        es_oh = [keep.tile([128, NSEG], bf16, name=f"esoh_{kc}")
                 for kc in range(KT)]
        es_st = keep.tile([128, KT, NST], bf16, name="esst")
        ef_b = keep.tile([128, KT, EF], bf16, name="efb")

        # ---- targets front (DVE): argmax straight off raw targets ----
        t1big = keep.tile([128, KT, D * C], f32, name="t1big")
        nc.vector.tensor_scalar_add(out=t1big[:], in0=tbig[:], scalar1=1e-10)
        t1v = t1big[:].rearrange("p a (d c) -> p (a d) c", c=C)
        mxall = keep.tile([128, KT * D], f32, name="mxall")
        nc.vector.reduce_max(out=mxall[:], in_=t1v, axis=Ax.X)
        invsb = keep.tile([128, KT * D], f32, name="invsb")
        nc.vector.reduce_sum(out=invsb[:], in_=t1v, axis=Ax.X)
        nc.vector.reciprocal(invsb[:], invsb[:])

        # ---- row sum-of-squares on DVE; sqrt on ACT (table prefetched) --
        sqsc = keep.tile([128, 2, FD], f32, name="sqsc")
        sqpack = keep.tile([128, KT], f32, name="sqpack")
        nc.vector.tensor_tensor(out=sqsc[:], in0=xbig0[:], in1=xbig0[:],
                                op=Alu.mult)
        nc.vector.reduce_sum(out=sqpack[:, 0:2], in_=sqsc[:], axis=Ax.X)
        nc.vector.tensor_tensor(out=sqsc[:], in0=xbig1[:], in1=xbig1[:],
                                op=Alu.mult)
        nc.vector.reduce_sum(out=sqpack[:, 2:4], in_=sqsc[:], axis=Ax.X)
        normpack = keep.tile([128, KT], f32, name="normpack")
        sqrt_act = nc.scalar.sqrt(normpack[:], sqpack[:])
        nc.vector.tensor_scalar_max(out=normpack[:], in0=normpack[:],
                                    scalar1=1e-12)
        invpack = keep.tile([128, KT], f32, name="invpack")
        nc.vector.reciprocal(invpack[:], normpack[:])
        minvpack = keep.tile([128, KT], f32, name="minvpack")
        nc.vector.tensor_tensor(out=minvpack[:], in0=invpack[:],
                                in1=mkbig[:, :, 0], op=Alu.mult)

        # ---- ext_feat = [x*(m/||x||) | m] bf16 (ACT, tableless copies) --
        copy_acts = []
        for kc in range(KT):
            copy_acts.append(nc.scalar.activation(
                out=ef_b[:, kc, 0:FD], in_=xchunk(kc), func=Act.Copy,
                scale=minvpack[:, kc:kc + 1]))
        nc.vector.tensor_copy(out=ef_b[:, :, FD:FD + 1], in_=mkbig[:])

        # ---- argmax -> code -> one-hot (DVE) ----
        candall = keep.tile([128, KT * D, C], f32, name="candall")
        for g in range(KT * D):
            nc.vector.scalar_tensor_tensor(
                out=candall[:, g, :], in0=t1v[:, g, :],
                scalar=mxall[:, g:g + 1], in1=biota[:],
                op0=Alu.is_equal, op1=Alu.mult)
        mqall = keep.tile([128, KT * D], f32, name="mqall")
        nc.vector.reduce_max(out=mqall[:], in_=candall[:], axis=Ax.X)
        clsall = keep.tile([128, KT * D], f32, name="clsall")
        nc.vector.tensor_scalar(out=clsall[:], in0=mqall[:], scalar1=-1.0,
                                scalar2=float(C), op0=Alu.mult, op1=Alu.add)
        clsv = clsall[:].rearrange("p (a two) -> p a two", two=2)
        codeall = keep.tile([128, KT], f32, name="codeall")
        nc.vector.scalar_tensor_tensor(
            out=codeall[:], in0=clsv[:, :, 1], scalar=float(C),
            in1=clsv[:, :, 0], op0=Alu.mult, op1=Alu.add)
        for kc in range(KT):
            nc.vector.tensor_scalar(
                out=es_oh[kc][:], in0=iota1024[:],
                scalar1=codeall[:, kc:kc + 1],
                scalar2=None, op0=Alu.is_equal)

        # ---- stats columns [lq | p | ones | E]: p written by the muls,
        # lq by one batched Ln (ACT), E from the bf16 slices ----
        for g in range(KT * D):
            kc, d_ = divmod(g, D)
            nc.vector.tensor_scalar_mul(
                out=es_st[:, kc, D * C + C * d_:D * C + C * (d_ + 1)],
                in0=t1v[:, g, :], scalar1=invsb[:, g:g + 1])
        ln_act = nc.scalar.activation(out=es_st[:, :, 0:D * C],
                                      in_=es_st[:, :, D * C:2 * D * C],
                                      func=Act.Ln)
        scrall = keep.tile([128, KT, D * C], f32, name="scrall")
        nc.vector.tensor_tensor(out=scrall[:], in0=es_st[:, :, D * C:2 * D * C],
                                in1=es_st[:, :, 0:D * C], op=Alu.mult)
        ecolall = keep.tile([128, KT * D], f32, name="ecolall")
        nc.vector.reduce_sum(
            out=ecolall[:],
            in_=scrall[:].rearrange("p a (d c) -> p (a d) c", c=C),
            axis=Ax.X)
        ecol2 = keep.tile([128, KT], f32, name="ecol2")
        nc.vector.reduce_sum(
            out=ecol2[:],
            in_=ecolall[:].rearrange("p (a d) -> p a d", d=D),
            axis=Ax.X)
        nc.vector.tensor_copy(out=es_st[:, :, NST - 1:NST],
                              in_=ecol2[:].rearrange("p a -> p a ()"))
        nc.vector.memset(es_st[:, :, NST - 2:NST - 1], 1.0)

        # Square-table prefetch for the epilogue, at the ACT chain's end
        pre2 = consts.tile([1, 1], f32)
        pre2_act = nc.scalar.activation(out=pre2[:], in_=ones128[0:1, 0:1],
                                        func=Act.Square)

        act_chain = [pre_act, sqrt_act] + copy_acts + [ln_act, pre2_act]
        for a, b in zip(act_chain[1:], act_chain[:-1]):
            add_dep_helper(a.ins, b.ins, sync=False,
                           reason="act table grouping")

        # ---------- matmul: kc-outer, m-tiles 0-7 in all 8 banks --------
        resab = keep.tile([128, 8, EF], fp16, name="resab")
        ps8 = [psum.tile([128, EF], f32, name=f"ps_{mt}", tag=f"ps_{mt}")
               for mt in range(8)]
        for kc in range(KT):
            for mt in range(8):
                nc.tensor.matmul(out=ps8[mt][:],
                                 lhsT=es_oh[kc][:, mt * 128:(mt + 1) * 128],
                                 rhs=ef_b[:, kc, :],
                                 start=(kc == 0), stop=(kc == KT - 1))
        # banks 0/1 cast first so the trailing stats wave can start
        nc.vector.tensor_copy(out=resab[:, 0, :], in_=ps8[0][:])
        nc.scalar.activation(out=resab[:, 1, :], in_=ps8[1][:], func=Act.Copy)
        for mt in range(2, 8):
            if mt % 2 == 0:
                nc.vector.tensor_copy(out=resab[:, mt, :], in_=ps8[mt][:])
            else:
                nc.scalar.activation(out=resab[:, mt, :], in_=ps8[mt][:],
                                     func=Act.Copy)
        nc.sync.dma_start(
            out=inbounce[:, 0:8 * EF].rearrange("p (a f) -> p a f", f=EF),
            in_=resab[:])

        # trailing wave: stats m-tiles 8 (LQ|P) and 9 (ones|E)
        psC0 = psum.tile([128, EF], f32, name="psC0", tag="ps_0")
        psC1 = psum.tile([2, EF], f32, name="psC1", tag="ps_1")
        for kc in range(KT):
            nc.tensor.matmul(out=psC0[:], lhsT=es_st[:, kc, 0:128],
                             rhs=ef_b[:, kc, :],
                             start=(kc == 0), stop=(kc == KT - 1))
            nc.tensor.matmul(out=psC1[:], lhsT=es_st[:, kc, 128:130],
                             rhs=ef_b[:, kc, :],
                             start=(kc == 0), stop=(kc == KT - 1))
        resc0 = keep.tile([128, EF], fp16, name="resc0")
        nc.vector.tensor_copy(out=resc0[:], in_=psC0[:])
        nc.scalar.dma_start(
            out=inbounce[:, 8 * EF:9 * EF], in_=resc0[:])
        # tail rows flattened: row r -> partitions 64r..64r+64, 5 cols each
        resc1 = keep.tile([2, 64 * TAILW], fp16, name="resc1")
        nc.vector.memset(resc1[:], 0.0)
        nc.vector.tensor_copy(out=resc1[:, 0:EF], in_=psC1[:])
        nc.scalar.dma_start(
            out=inbounce[:, TW:BW].rearrange("(p a) f -> p a f", a=64),
            in_=resc1[:].rearrange("p (a f) -> p a f", f=TAILW))

        # ---------------- single AllReduce (fp16) ----------------
        nc.gpsimd.collective_compute(
            "AllReduce", mybir.AluOpType.add,
            replica_groups=[list(range(NCORES))],
            ins=[inbounce.opt()], outs=[outbounce.opt()])

        # ---------------- epilogue (redundant on every core) ------------
        bigall = keep.tile([128, 8, EF], fp16, name="bigall")
        nc.sync.dma_start(
            out=bigall[:, 4:8, :],
            in_=outbounce[:, 4 * EF:8 * EF].rearrange("p (a f) -> p a f",
                                                      f=EF))
        nc.gpsimd.dma_start(
            out=bigall[:, 0:4, :],
            in_=outbounce[:, 0:4 * EF].rearrange("p (a f) -> p a f", f=EF))
        uv2 = keep.tile([64, 2, EF], fp16, name="uv2")
        nc.scalar.dma_start(
            out=uv2[:],
            in_=outbounce[:, 8 * EF:9 * EF].rearrange(
                "(two r) f -> r two f", two=2))
        # tail rows: ones row on partitions 0:64, E row on 64:128
        tail2 = keep.tile([1, 2, 64 * TAILW], fp16, name="tail2")
        nc.scalar.dma_start(
            out=tail2[:].rearrange("q two (p f) -> q two p f", f=TAILW),
            in_=outbounce[:, TW:BW].rearrange("(two p) f -> () two p f",
                                              two=2))
        onesrow = tail2[0:1, 0, :]
        erow = tail2[0:1, 1, :]

        Z = keep.tile([128, 8], f32, name="Z")
        nc.vector.memset(Z[:], 0.0)
        nrmp = keep.tile([128, 8], f32, name="nrmp")
        # segment squares: slots 0-3 on DVE, 4-7 on ACT (prefetched table)
        scr4 = keep.tile([128, 4, FD], f32, name="scr4")
        nc.vector.tensor_tensor(out=scr4[:], in0=bigall[:, 0:4, 0:FD],
                                in1=bigall[:, 0:4, 0:FD], op=Alu.mult)
        nc.vector.reduce_sum(out=nrmp[:, 0:4], in_=scr4[:], axis=Ax.X)
        scrsq = keep.tile([128, FD], f32, name="scrsq")
        sq_acts = []
        for sl in range(4, 8):
            sq_acts.append(nc.scalar.activation(
                out=scrsq[:], in_=bigall[:, sl, 0:FD], func=Act.Square,
                accum_out=nrmp[:, sl:sl + 1]))
        for a, b in zip(sq_acts, [pre2_act] + sq_acts[:-1]):
            add_dep_helper(a.ins, b.ins, sync=False,
                           reason="act table grouping")
        cdp = keep.tile([128, 8], f32, name="cdp")
        nc.vector.tensor_scalar_max(out=cdp[:], in0=bigall[:, :, FD],
                                    scalar1=1.0)
        rcdp = keep.tile([128, 8], f32, name="rcdp")
        nc.vector.reciprocal(rcdp[:], cdp[:])
        termp = keep.tile([128, 8], f32, name="termp")
        nc.vector.tensor_tensor(out=termp[:], in0=nrmp[:], in1=rcdp[:],
                                op=Alu.mult)
        nc.vector.reduce_sum(out=Z[:, 0:1], in_=termp[:], axis=Ax.X)

        scrU = keep.tile([64, FD], f32, name="scrU")
        nc.vector.tensor_tensor(out=scrU[:], in0=uv2[:, 0, 0:FD],
                                in1=uv2[:, 1, 0:FD], op=Alu.mult)
        nc.vector.reduce_sum(out=Z[0:64, 1:2], in_=scrU[:], axis=Ax.X)
        nc.vector.scalar_tensor_tensor(
            out=Z[0:64, 2:3], in0=uv2[:, 0, FD:FD + 1], scalar=-1.0,
            in1=uv2[:, 1, FD:FD + 1], op0=Alu.mult, op1=Alu.mult)
        scrF = keep.tile([1, FD], f32, name="scrF")
        nc.vector.scalar_tensor_tensor(
            out=scrF[:], in0=onesrow[:, 0:FD], scalar=-1.0,
            in1=erow[:, 0:FD], op0=Alu.mult, op1=Alu.mult)
        nc.vector.reduce_sum(out=Z[0:1, 4:5], in_=scrF[:], axis=Ax.X)  # -Fe.F~

        zred = psum.tile([1, 8], f32, name="zred", tag="ps_2")
        nc.tensor.matmul(out=zred[:], lhsT=ones128[:], rhs=Z[:],
                         start=True, stop=True)
        zs = keep.tile([1, 8], f32, name="zs")
        nc.vector.tensor_copy(out=zs[:], in_=zred[:])

        Mv = keep.tile([1, 2], f32, name="Mv")
        nc.vector.tensor_copy(out=Mv[0:1, 0:1], in_=onesrow[:, FD:FD + 1])
        nc.vector.tensor_copy(out=Mv[0:1, 1:2], in_=erow[:, FD:FD + 1])
        Ms = Mv[0:1, 0:1]
        ev = Mv[0:1, 1:2]
        s_center = zs[0:1, 0:1]
        uv = zs[0:1, 1:2]
        pl = zs[0:1, 2:3]
        fef = zs[0:1, 4:5]

        fin = keep.tile([1, 16], f32, name="fin")
        t_ = lambda i: fin[0:1, i:i + 1]
        # off-chain scalars (ready as soon as tail2 lands)
        nc.vector.tensor_tensor(out=t_(8), in0=Ms, in1=ev, op=Alu.mult)  # M*e
        nc.vector.tensor_scalar(out=t_(15), in0=Ms, scalar1=-1.0,
                                scalar2=Ms, op0=Alu.add, op1=Alu.mult)
        nc.vector.reciprocal(t_(15), t_(15))        # 1/(M*(M-1))
        nc.vector.reciprocal(t_(6), Ms)             # 1/M
        # zsum = uv - pl - fef  (cols 1..4 of zs; col 3 is zero)
        nc.vector.reduce_sum(out=t_(9), in_=zs[0:1, 1:5], axis=Ax.X)
        nc.vector.tensor_tensor(out=t_(11), in0=t_(8), in1=t_(9), op=Alu.add)
        nc.vector.tensor_tensor(out=t_(12), in0=t_(11), in1=t_(15),
                                op=Alu.mult)
        nc.vector.tensor_scalar_mul(out=t_(1), in0=t_(12), scalar1=-1.0)
        nc.vector.tensor_tensor(out=t_(7), in0=s_center, in1=t_(6),
                                op=Alu.mult)
        nc.vector.tensor_scalar(out=t_(2), in0=t_(7), scalar1=-1.0,
                                scalar2=1.0, op0=Alu.mult, op1=Alu.add)
        # total = 0.1*(1 - s7 - t12) = -0.1*(s7 + t12) + 0.1
        nc.vector.tensor_tensor(out=t_(13), in0=t_(7), in1=t_(12), op=Alu.add)
        nc.vector.tensor_scalar(out=t_(0), in0=t_(13), scalar1=-0.1,
                                scalar2=0.1, op0=Alu.mult, op1=Alu.add)
        # debug slots
        nc.vector.tensor_copy(out=t_(3), in_=Ms)
        nc.vector.tensor_copy(out=t_(4), in_=ev)
        nc.vector.tensor_copy(out=t_(5), in_=uv)

        nc.scalar.dma_start(out=outd[None, :], in_=fin[0:1, 0:8])

    nc.finalize()
    return nc


def _get_compiled():
    if "nc" not in _compiled:
        _compiled["nc"] = _build_bass()
    return _compiled["nc"]


def _make_in_maps(features, targets, mask):
    features = np.ascontiguousarray(np.asarray(features, dtype=np.float32))
    targets = np.ascontiguousarray(np.asarray(targets, dtype=np.float32))
    maskf = np.asarray(mask).astype(np.float32).reshape(B, 1)
    in_maps = []
    for i in range(NCORES):
        sl = slice(i * RB, (i + 1) * RB)
        in_maps.append({
            "features": features[sl],
            "targets": targets[sl],
            "maskf": np.ascontiguousarray(maskf[sl]),
        })
    return in_maps


def kernel(features, targets, mask):
    from concourse.bass_utils import run_bass_kernel_spmd

    nc = _get_compiled()
    in_maps = _make_in_maps(features, targets, mask)
    res = run_bass_kernel_spmd(nc, in_maps, list(range(NCORES)))
    out = res.results[0]["out"]
    total = np.float32(out[0])
    diversity = np.float32(out[1])
    tightness = np.float32(out[2])
    return total, diversity, tightness
